# revision 5
# baseline (speedup 1.0000x reference)
"""Bahdanau additive attention on Trainium2, SPMD over 8 NeuronCores.

Problem (per batch element b):
    q_proj = query @ Ws.T            (T, H)
    e_proj = enc   @ Wh.T            (S, H)
    scores[t, s] = sum_h v[h] * tanh(q_proj[t, h] + e_proj[s, h])
    attn = masked softmax over s     (mask: s < src_lengths[b])
    out[t, h] = sum_s attn[t, s] * enc[s, h]

Sharding: data-parallel over B=8, one batch element per core. No
collectives.

Per-core dataflow (feature dim H lives as 4 o-tiles of 128 partitions):
  - PE: q_projT[o, t], e_projT[o, s] via matmuls on pre-transposed
    host inputs.
  - main loop over t in blocks of TB: DVE tensor_scalar_add broadcasts
    q_projT[:, t] (per-partition scalar) over e_projT -> tanh input;
    one big ACT Tanh per block ([128, TB*4*256] free dim amortizes the
    ~224-cycle ACT instruction overhead) writing bf16; PE uses each
    tanh [128 o, 128 s] slice as the stationary operand (bf16 enables
    fast weight load) against moving v[:, j] [128, 1], accumulating a
    [128 s, 1] column into scoresT PSUM tiles at free offset t.
  - softmax in the transposed layout: ACT Exp psum->sbuf; mask is a
    per-partition scalar multiply; denominator via ones-matmul
    (reduce over s partitions) + tiny transpose matmul to get a
    [t, 1] column; context = expT (unnormalized) as lhsT against
    enc[s, h], normalization folded into the PSUM->SBUF copy.
"""

from contextlib import ExitStack

import numpy as np

import concourse.bass as bass
import concourse.bacc as bacc
import concourse.mybir as mybir
import concourse.tile as tile
from concourse.bass_utils import run_bass_kernel_spmd

B, T, S, H = 8, 128, 256, 512
NCORES = 8
P = 128          # partitions
KT = H // P      # 4 feature tiles
ST = S // P      # 2 source tiles
TB = 4           # t-block size for ACT batching

dt = mybir.dt
AF = mybir.ActivationFunctionType


def _build_kernel(tc: tile.TileContext, ctx: ExitStack, aps: dict):
    nc = tc.nc
    f32 = dt.float32
    bf16 = dt.bfloat16

    const = ctx.enter_context(tc.tile_pool(name="const", bufs=1))
    psA = ctx.enter_context(tc.tile_pool(name="psA", bufs=1, space="PSUM"))

    # ---- load inputs ------------------------------------------------
    wsT_sb = const.tile([P, KT, H], f32)
    nc.sync.dma_start(wsT_sb[:], aps["WsT"].rearrange("(k p) o -> p k o", p=P))
    whT_sb = const.tile([P, KT, H], f32)
    nc.sync.dma_start(whT_sb[:], aps["WhT"].rearrange("(k p) o -> p k o", p=P))
    queryT_sb = const.tile([P, KT, T], f32)
    nc.sync.dma_start(queryT_sb[:], aps["queryT"].rearrange("(k p) t -> p k t", p=P))
    encT_sb = const.tile([P, KT, S], f32)
    nc.sync.dma_start(encT_sb[:], aps["encT"].rearrange("(k p) s -> p k s", p=P))
    enc_sb = const.tile([P, ST, H], f32)
    nc.sync.dma_start(enc_sb[:], aps["enc"].rearrange("(u p) h -> p u h", p=P))
    vcol_sb = const.tile([P, KT], bf16)
    nc.sync.dma_start(vcol_sb[:], aps["vcol"][:, :])
    maskT_sb = const.tile([P, ST], f32)
    nc.sync.dma_start(maskT_sb[:], aps["maskT"][:, :])

    # ---- projections ------------------------------------------------
    # q_projT[o, t] = sum_h Ws[o, h] * query[t, h]
    q_projT_sb = const.tile([P, KT, T], f32)
    for j in range(KT):
        qp_ps = psA.tile([P, T], f32, tag="qp")
        for k in range(KT):
            nc.tensor.matmul(
                qp_ps[:],
                lhsT=wsT_sb[:, k, j * P:(j + 1) * P],
                rhs=queryT_sb[:, k, :],
                start=(k == 0),
                stop=(k == KT - 1),
            )
        nc.vector.tensor_copy(q_projT_sb[:, j, :], qp_ps[:])

    # e_projT[o, s] = sum_h Wh[o, h] * enc[s, h]
    e_projT_sb = const.tile([P, KT, S], f32)
    for j in range(KT):
        ep_ps = psA.tile([P, S], f32, tag="ep")
        for k in range(KT):
            nc.tensor.matmul(
                ep_ps[:],
                lhsT=whT_sb[:, k, j * P:(j + 1) * P],
                rhs=encT_sb[:, k, :],
                start=(k == 0),
                stop=(k == KT - 1),
            )
        nc.vector.tensor_copy(e_projT_sb[:, j, :], ep_ps[:])

    # ---- main loop: scoresT[s, t] in PSUM ---------------------------
    scT_pool = ctx.enter_context(tc.tile_pool(name="scT", bufs=1, space="PSUM"))
    scT_ps = [scT_pool.tile([P, T], f32, tag=f"scT{u}", name=f"scT{u}") for u in range(ST)]
    tanh_pool = ctx.enter_context(tc.tile_pool(name="tanh", bufs=2))

    for tb in range(T // TB):
        t0 = tb * TB
        tin = tanh_pool.tile([P, TB, KT, S], f32, tag="tin")
        tout = tanh_pool.tile([P, TB, KT, S], bf16, tag="tout")
        for tl in range(TB):
            for j in range(KT):
                nc.vector.tensor_scalar_add(
                    tin[:, tl, j, :],
                    e_projT_sb[:, j, :],
                    q_projT_sb[:, j, t0 + tl:t0 + tl + 1],
                )
        nc.scalar.activation(tout[:], tin[:], AF.Tanh)
        for tl in range(TB):
            t = t0 + tl
            for u in range(ST):
                for j in range(KT):
                    nc.tensor.matmul(
                        scT_ps[u][:, t:t + 1],
                        lhsT=tout[:, tl, j, u * P:(u + 1) * P],
                        rhs=vcol_sb[:, j:j + 1],
                        start=(j == 0),
                        stop=(j == KT - 1),
                    )

    # ---- masked softmax over s (s on partitions) --------------------
    expT_sb = const.tile([P, ST, T], f32)
    for u in range(ST):
        nc.scalar.activation(expT_sb[:, u, :], scT_ps[u][:], AF.Exp)
        nc.vector.tensor_scalar_mul(
            expT_sb[:, u, :], expT_sb[:, u, :], maskT_sb[:, u:u + 1]
        )

    ones_sb = const.tile([P, 1], f32)
    nc.vector.memset(ones_sb[:], 1.0)
    den_ps = psA.tile([1, T], f32, tag="den")
    for u in range(ST):
        nc.tensor.matmul(
            den_ps[:],
            lhsT=ones_sb[:],
            rhs=expT_sb[:, u, :],
            start=(u == 0),
            stop=(u == ST - 1),
        )
    den_row_sb = const.tile([1, T], f32)
    nc.vector.tensor_copy(den_row_sb[:], den_ps[:])
    one1_sb = const.tile([1, 1], f32)
    nc.vector.memset(one1_sb[:], 1.0)
    den_col_ps = psA.tile([P, 1], f32, tag="denc")
    nc.tensor.matmul(den_col_ps[:], lhsT=den_row_sb[:], rhs=one1_sb[:])
    rden_sb = const.tile([P, 1], f32)
    nc.vector.reciprocal(rden_sb[:], den_col_ps[:])

    # ---- context: out[t, h] = sum_s expT[s, t] * enc[s, h] / den[t] --
    ctx_ps = psA.tile([P, H], f32, tag="ctx")
    for u in range(ST):
        nc.tensor.matmul(
            ctx_ps[:],
            lhsT=expT_sb[:, u, :],
            rhs=enc_sb[:, u, :],
            start=(u == 0),
            stop=(u == ST - 1),
        )
    ctx_sb = const.tile([P, H], f32)
    nc.vector.tensor_scalar_mul(ctx_sb[:], ctx_ps[:], rden_sb[:])
    nc.sync.dma_start(aps["out"][:, :], ctx_sb[:])


def build_nc() -> bass.Bass:
    nc = bacc.Bacc("TRN2", target_bir_lowering=False, debug=False)
    aps = {
        "queryT": nc.dram_tensor("queryT", [H, T], dt.float32, kind="ExternalInput").ap(),
        "encT": nc.dram_tensor("encT", [H, S], dt.float32, kind="ExternalInput").ap(),
        "enc": nc.dram_tensor("enc", [S, H], dt.float32, kind="ExternalInput").ap(),
        "WsT": nc.dram_tensor("WsT", [H, H], dt.float32, kind="ExternalInput").ap(),
        "WhT": nc.dram_tensor("WhT", [H, H], dt.float32, kind="ExternalInput").ap(),
        "vcol": nc.dram_tensor("vcol", [P, KT], dt.bfloat16, kind="ExternalInput").ap(),
        "maskT": nc.dram_tensor("maskT", [P, ST], dt.float32, kind="ExternalInput").ap(),
        "out": nc.dram_tensor("out", [T, H], dt.float32, kind="ExternalOutput").ap(),
    }
    with ExitStack() as ctx:
        with tile.TileContext(nc) as tc:
            _build_kernel(tc, ctx, aps)
            ctx.close()
    nc.compile()
    return nc


def make_in_maps(query, encoder_outputs, src_lengths, Ws, Wh, v):
    import ml_dtypes

    wsT = np.ascontiguousarray(Ws.T).astype(np.float32)
    whT = np.ascontiguousarray(Wh.T).astype(np.float32)
    vcol = np.ascontiguousarray(
        np.asarray(v, np.float32).reshape(KT, P).T
    ).astype(ml_dtypes.bfloat16)
    in_maps = []
    for b in range(B):
        m01 = (np.arange(S) < int(src_lengths[b])).astype(np.float32)
        maskT = np.ascontiguousarray(m01.reshape(ST, P).T)  # [P, ST]
        in_maps.append({
            "queryT": np.ascontiguousarray(np.asarray(query[b], np.float32).T),
            "encT": np.ascontiguousarray(np.asarray(encoder_outputs[b], np.float32).T),
            "enc": np.ascontiguousarray(np.asarray(encoder_outputs[b], np.float32)),
            "WsT": wsT,
            "WhT": whT,
            "vcol": vcol,
            "maskT": maskT,
        })
    return in_maps


_NC_CACHE = None


def kernel(query, encoder_outputs, src_lengths, Ws, Wh, v):
    global _NC_CACHE
    if _NC_CACHE is None:
        _NC_CACHE = build_nc()
    nc = _NC_CACHE
    in_maps = make_in_maps(query, encoder_outputs, src_lengths, Ws, Wh, v)
    res = run_bass_kernel_spmd(nc, in_maps, core_ids=list(range(NCORES)))
    out = np.stack([res.results[b]["out"] for b in range(B)], axis=0)
    return out.astype(np.float32)


# revision 6
# speedup vs baseline: 1.0897x; 1.0897x over previous
"""Bahdanau additive attention on Trainium2, SPMD over 8 NeuronCores.

Problem (per batch element b):
    q_proj = query @ Ws.T            (T, H)
    e_proj = enc   @ Wh.T            (S, H)
    scores[t, s] = sum_h v[h] * tanh(q_proj[t, h] + e_proj[s, h])
    attn = masked softmax over s     (mask: s < src_lengths[b])
    out[t, h] = sum_s attn[t, s] * enc[s, h]

Sharding: data-parallel over B=8, one batch element per core. No
collectives.

Per-core dataflow (feature dim H lives as 4 o-tiles of 128 partitions):
  - PE: q_projT[o, t], e_projT[o, s] via matmuls on pre-transposed
    host inputs.
  - main loop over t in blocks of TB: DVE tensor_scalar_add broadcasts
    q_projT[:, t] (per-partition scalar) over e_projT -> tanh input;
    one big ACT Tanh per block ([128, TB*4*256] free dim amortizes the
    ~224-cycle ACT instruction overhead) writing bf16; PE uses each
    tanh [128 o, 128 s] slice as the stationary operand (bf16 enables
    fast weight load) against moving v[:, j] [128, 1], accumulating a
    [128 s, 1] column into scoresT PSUM tiles at free offset t.
  - softmax in the transposed layout: ACT Exp psum->sbuf; mask is a
    per-partition scalar multiply; denominator via ones-matmul
    (reduce over s partitions) + tiny transpose matmul to get a
    [t, 1] column; context = expT (unnormalized) as lhsT against
    enc[s, h], normalization folded into the PSUM->SBUF copy.
"""

from contextlib import ExitStack

import numpy as np

import concourse.bass as bass
import concourse.bacc as bacc
import concourse.mybir as mybir
import concourse.tile as tile
from concourse.bass_utils import run_bass_kernel_spmd

B, T, S, H = 8, 128, 256, 512
NCORES = 8
P = 128          # partitions
KT = H // P      # 4 feature tiles
ST = S // P      # 2 source tiles
TB = 8           # t-block size for ACT batching

dt = mybir.dt
AF = mybir.ActivationFunctionType


def _build_kernel(tc: tile.TileContext, ctx: ExitStack, aps: dict):
    nc = tc.nc
    f32 = dt.float32
    bf16 = dt.bfloat16
    f16 = dt.float16

    const = ctx.enter_context(tc.tile_pool(name="const", bufs=1))
    psA = ctx.enter_context(tc.tile_pool(name="psA", bufs=1, space="PSUM"))

    # ---- load inputs ------------------------------------------------
    wsT_sb = const.tile([P, KT, H], f32)
    whT_sb = const.tile([P, KT, H], f32)
    queryT_sb = const.tile([P, KT, T], f32)
    encT_sb = const.tile([P, KT, S], f32)
    wsT_r = aps["WsT"].rearrange("(k p) o -> k p o", p=P)
    whT_r = aps["WhT"].rearrange("(k p) o -> k p o", p=P)
    queryT_r = aps["queryT"].rearrange("(k p) t -> k p t", p=P)
    encT_r = aps["encT"].rearrange("(k p) s -> k p s", p=P)
    for k in range(KT):
        nc.sync.dma_start(queryT_sb[:, k, :], queryT_r[k])
        nc.sync.dma_start(encT_sb[:, k, :], encT_r[k])
        nc.sync.dma_start(wsT_sb[:, k, :], wsT_r[k])
        nc.sync.dma_start(whT_sb[:, k, :], whT_r[k])
    enc_sb = const.tile([P, ST, H], f32)
    enc_r = aps["enc"].rearrange("(u p) h -> u p h", p=P)
    for u in range(ST):
        nc.sync.dma_start(enc_sb[:, u, :], enc_r[u])
    vcol_sb = const.tile([P, KT], f16)
    nc.sync.dma_start(vcol_sb[:], aps["vcol"][:, :])
    maskT_sb = const.tile([P, ST], f32)
    nc.sync.dma_start(maskT_sb[:], aps["maskT"][:, :])

    # ---- projections ------------------------------------------------
    # q_projT[o, t] = sum_h Ws[o, h] * query[t, h]
    q_projT_sb = const.tile([P, KT, T], f32)
    for j in range(KT):
        qp_ps = psA.tile([P, T], f32, tag="qp")
        for k in range(KT):
            nc.tensor.matmul(
                qp_ps[:],
                lhsT=wsT_sb[:, k, j * P:(j + 1) * P],
                rhs=queryT_sb[:, k, :],
                start=(k == 0),
                stop=(k == KT - 1),
            )
        nc.vector.tensor_copy(q_projT_sb[:, j, :], qp_ps[:])

    # e_projT[o, s] = sum_h Wh[o, h] * enc[s, h]
    e_projT_sb = const.tile([P, KT, S], f16)
    for j in range(KT):
        ep_ps = psA.tile([P, S], f32, tag="ep")
        for k in range(KT):
            nc.tensor.matmul(
                ep_ps[:],
                lhsT=whT_sb[:, k, j * P:(j + 1) * P],
                rhs=encT_sb[:, k, :],
                start=(k == 0),
                stop=(k == KT - 1),
            )
        nc.vector.tensor_copy(e_projT_sb[:, j, :], ep_ps[:])

    # ---- main loop: scoresT[s, t] in PSUM ---------------------------
    scT_pool = ctx.enter_context(tc.tile_pool(name="scT", bufs=1, space="PSUM"))
    scT_ps = [scT_pool.tile([P, T], f32, tag=f"scT{u}", name=f"scT{u}") for u in range(ST)]
    tanh_pool = ctx.enter_context(tc.tile_pool(name="tanh", bufs=2))

    for tb in range(T // TB):
        t0 = tb * TB
        tin = tanh_pool.tile([P, TB, KT, S], f16, tag="tin")
        tout = tanh_pool.tile([P, TB, KT, S], f16, tag="tout")
        for tl in range(TB):
            for j in range(KT):
                nc.vector.tensor_scalar_add(
                    tin[:, tl, j, :],
                    e_projT_sb[:, j, :],
                    q_projT_sb[:, j, t0 + tl:t0 + tl + 1],
                )
        nc.scalar.activation(tout[:], tin[:], AF.Tanh)
        for tl in range(TB):
            t = t0 + tl
            for u in range(ST):
                for j in range(KT):
                    nc.tensor.matmul(
                        scT_ps[u][:, t:t + 1],
                        lhsT=tout[:, tl, j, u * P:(u + 1) * P],
                        rhs=vcol_sb[:, j:j + 1],
                        start=(j == 0),
                        stop=(j == KT - 1),
                    )

    # ---- masked softmax over s (s on partitions) --------------------
    expT_sb = const.tile([P, ST, T], f32)
    for u in range(ST):
        nc.scalar.activation(expT_sb[:, u, :], scT_ps[u][:], AF.Exp)
        nc.vector.tensor_scalar_mul(
            expT_sb[:, u, :], expT_sb[:, u, :], maskT_sb[:, u:u + 1]
        )

    ones_sb = const.tile([P, 1], f32)
    nc.vector.memset(ones_sb[:], 1.0)
    den_ps = psA.tile([1, T], f32, tag="den")
    for u in range(ST):
        nc.tensor.matmul(
            den_ps[:],
            lhsT=ones_sb[:],
            rhs=expT_sb[:, u, :],
            start=(u == 0),
            stop=(u == ST - 1),
        )
    den_row_sb = const.tile([1, T], f32)
    nc.vector.tensor_copy(den_row_sb[:], den_ps[:])
    one1_sb = const.tile([1, 1], f32)
    nc.vector.memset(one1_sb[:], 1.0)
    den_col_ps = psA.tile([P, 1], f32, tag="denc")
    nc.tensor.matmul(den_col_ps[:], lhsT=den_row_sb[:], rhs=one1_sb[:])
    rden_sb = const.tile([P, 1], f32)
    nc.vector.reciprocal(rden_sb[:], den_col_ps[:])

    # ---- context: out[t, h] = sum_s expT[s, t] * enc[s, h] / den[t] --
    ctx_ps = psA.tile([P, H], f32, tag="ctx")
    for u in range(ST):
        nc.tensor.matmul(
            ctx_ps[:],
            lhsT=expT_sb[:, u, :],
            rhs=enc_sb[:, u, :],
            start=(u == 0),
            stop=(u == ST - 1),
        )
    ctx_sb = const.tile([P, H], f32)
    nc.vector.tensor_scalar_mul(ctx_sb[:], ctx_ps[:], rden_sb[:])
    nc.sync.dma_start(aps["out"][:, :], ctx_sb[:])


def build_nc() -> bass.Bass:
    nc = bacc.Bacc("TRN2", target_bir_lowering=False, debug=False)
    aps = {
        "queryT": nc.dram_tensor("queryT", [H, T], dt.float32, kind="ExternalInput").ap(),
        "encT": nc.dram_tensor("encT", [H, S], dt.float32, kind="ExternalInput").ap(),
        "enc": nc.dram_tensor("enc", [S, H], dt.float32, kind="ExternalInput").ap(),
        "WsT": nc.dram_tensor("WsT", [H, H], dt.float32, kind="ExternalInput").ap(),
        "WhT": nc.dram_tensor("WhT", [H, H], dt.float32, kind="ExternalInput").ap(),
        "vcol": nc.dram_tensor("vcol", [P, KT], dt.float16, kind="ExternalInput").ap(),
        "maskT": nc.dram_tensor("maskT", [P, ST], dt.float32, kind="ExternalInput").ap(),
        "out": nc.dram_tensor("out", [T, H], dt.float32, kind="ExternalOutput").ap(),
    }
    with ExitStack() as ctx:
        with tile.TileContext(nc) as tc:
            _build_kernel(tc, ctx, aps)
            ctx.close()
    nc.compile()
    return nc


def make_in_maps(query, encoder_outputs, src_lengths, Ws, Wh, v):
    import ml_dtypes

    wsT = np.ascontiguousarray(Ws.T).astype(np.float32)
    whT = np.ascontiguousarray(Wh.T).astype(np.float32)
    vcol = np.ascontiguousarray(
        np.asarray(v, np.float32).reshape(KT, P).T
    ).astype(np.float16)
    in_maps = []
    for b in range(B):
        m01 = (np.arange(S) < int(src_lengths[b])).astype(np.float32)
        maskT = np.ascontiguousarray(m01.reshape(ST, P).T)  # [P, ST]
        in_maps.append({
            "queryT": np.ascontiguousarray(np.asarray(query[b], np.float32).T),
            "encT": np.ascontiguousarray(np.asarray(encoder_outputs[b], np.float32).T),
            "enc": np.ascontiguousarray(np.asarray(encoder_outputs[b], np.float32)),
            "WsT": wsT,
            "WhT": whT,
            "vcol": vcol,
            "maskT": maskT,
        })
    return in_maps


_NC_CACHE = None


def kernel(query, encoder_outputs, src_lengths, Ws, Wh, v):
    global _NC_CACHE
    if _NC_CACHE is None:
        _NC_CACHE = build_nc()
    nc = _NC_CACHE
    in_maps = make_in_maps(query, encoder_outputs, src_lengths, Ws, Wh, v)
    res = run_bass_kernel_spmd(nc, in_maps, core_ids=list(range(NCORES)))
    out = np.stack([res.results[b]["out"] for b in range(B)], axis=0)
    return out.astype(np.float32)
